# revision 26
# baseline (speedup 1.0000x reference)
"""Trainium2 Bass kernel for nn_BEM_50002009260181.

Module (B=4, L=1024, D=768, F=32):
    AKey   = tanh(A @ W_aup1.T + b_aup1)          (B,L,D)
    AValue = tan (A @ W_aup2.T + b_aup2)          (B,L,D)
    VKey   = tanh(V @ W_vup1.T + b_vup1)          (B,L,D)
    VValue = tanh(V @ W_vup2.T + b_vup2)          (B,L,D)
    TAQ    = tanh(T * (A @ w_a.T) + b_a)          (B,L,D)
    TVQ    = tanh(T * (V @ w_v.T) + b_v)          (B,L,D)
    ta     = softmax_L(sum_d TAQ*VKey)            (B,L)
    tv     = softmax_L(sum_d TVQ*AKey)            (B,L)
    out    = (AValue * ta[...,None], VValue * tv[...,None])

Sharding: 8 cores = (batch b, L-half h).  Each core computes the full-L
scores for its batch (duplicated across the 2 cores of a batch, avoiding
any cross-core communication for the softmax) and the outputs for its own
L-half.  Inputs are rotated per-core so the own half is always tiles 0-3.

Layout: L on partitions (8 l-tiles of 128), D on the free dim.  The
Linear(32->768) weights ride as rhs of K=33 matmuls (bias folded in via a
ones-row in the lhsT).  A-side operands live on partitions 0-32 and V-side
on 64-96, so A/V matmul pairs land in disjoint PE row-groups and execute
concurrently.  TAQ/TVQ are a single ACT op each (per-partition scale=q,
bias=b).  Score reductions are fused mul+reduce (STT accum).  Softmax
skips max-subtraction (|scores| < 40 << 88).  tan = sin/cos with sin via
add_range_wrap into [-pi,pi] and a Cody-Waite cos exact near the poles.
"""

import numpy as np

B, L, D, F = 4, 1024, 768, 32
NCORES = 8
LT = 128          # l-tile size (partition dim)
NT = L // LT      # 8 l-tiles per batch
NT_HALF = NT // 2 # 4 own tiles
K1 = F + 1        # contraction with bias row
VOFF = 64         # partition offset of the V-side operands

PI = float(np.pi)
PIO2_HI = float(np.float32(np.pi / 2))
PIO2_LO = float(np.float64(np.pi / 2) - np.float64(np.float32(np.pi / 2)))

_CACHE = {}


def _build():
    if "nc" in _CACHE:
        return _CACHE["nc"]

    import concourse.bacc as bacc
    from concourse import bass_isa
    import concourse.tile as tile
    import concourse.mybir as mybir

    F32 = mybir.dt.float32
    AF = mybir.ActivationFunctionType
    ALU = mybir.AluOpType

    nc = bacc.Bacc()

    # ---- DRAM I/O (per-core shapes) ----
    d_t = nc.dram_tensor("t_rot", [L, D], F32, kind="ExternalInput")
    # av_pack rows: 0:33 = [A.T ; ones], 64:97 = [V.T ; ones]
    d_av = nc.dram_tensor("av_pack", [VOFF + K1, L], F32, kind="ExternalInput")
    # rhs_pack rows 0:33 = [Wa1.T|ba1 , Wa2.T|ba2], rows 64:97 = [Wv1.T|bv1 , Wv2.T|bv2]
    d_rhs = nc.dram_tensor("rhs_pack", [VOFF + K1, 2 * D], F32, kind="ExternalInput")
    d_wq = nc.dram_tensor("w_q", [VOFF + F, 1], F32, kind="ExternalInput")
    d_b = nc.dram_tensor("b_ab", [LT, 2], F32, kind="ExternalInput")
    d_oa = nc.dram_tensor("out_a", [L // 2, D], F32, kind="ExternalOutput")
    d_ov = nc.dram_tensor("out_v", [L // 2, D], F32, kind="ExternalOutput")

    t_view = d_t.rearrange("(n p) d -> p n d", p=LT)     # [128, 8, 768]
    oa_view = d_oa.rearrange("(n p) d -> p n d", p=LT)   # [128, 4, 768]
    ov_view = d_ov.rearrange("(n p) d -> p n d", p=LT)

    with tile.TileContext(nc) as tc:
        with (
            tc.tile_pool(name="consts", bufs=1) as consts,
            tc.tile_pool(name="keys", bufs=2) as keys,
            tc.tile_pool(name="vals", bufs=1) as vals,
            tc.tile_pool(name="vwork", bufs=3) as vwork,
            tc.tile_pool(name="ps", bufs=1, space="PSUM") as ps,
        ):
            # ---- inputs into SBUF ----
            sb_wq = consts.tile([VOFF + F, 1], F32, tag="sb_wq")
            nc.sync.dma_start(out=sb_wq[:], in_=d_wq[:])
            sb_av = consts.tile([VOFF + K1, L], F32, tag="sb_av")
            nc.sync.dma_start(out=sb_av[0:K1, :], in_=d_av[0:K1, :])
            nc.sync.dma_start(out=sb_av[VOFF : VOFF + K1, :], in_=d_av[VOFF : VOFF + K1, :])
            sb_rhs = consts.tile([VOFF + K1, 2 * D], F32, tag="sb_rhs")
            nc.sync.dma_start(out=sb_rhs[:, 0:D], in_=d_rhs[:, 0:D])
            sb_b = consts.tile([LT, 2], F32, tag="sb_b")
            nc.sync.dma_start(out=sb_b[:], in_=d_b[:])
            nc.sync.dma_start(out=sb_rhs[:, D : 2 * D], in_=d_rhs[:, D : 2 * D])
            t_all = consts.tile([LT, NT, D], F32, tag="t_all")
            nc.sync.dma_start(out=t_all[:, 0:1, :], in_=t_view[:, 0:1, :])
            nc.sync.dma_start(out=t_all[:, 1:4, :], in_=t_view[:, 1:4, :])
            nc.sync.dma_start(out=t_all[:, 4:NT, :], in_=t_view[:, 4:NT, :])

            sb_lo = consts.tile([LT, 1], F32, tag="sb_lo")
            nc.gpsimd.memset(sb_lo[:], PIO2_LO)
            # dummy first ACT op: pulls the tanh/exp table load to t=0 instead
            # of serializing it behind the first data-dependent activation
            warm = consts.tile([LT, 2], F32, tag="warm")
            nc.gpsimd.memset(warm[:], 0.0)
            nc.scalar.activation(out=warm[:, 1:2], in_=warm[:, 0:1], func=AF.Tanh)

            A0, A1 = 0, K1                  # a-side lhsT rows
            V0, V1 = VOFF, VOFF + K1        # v-side lhsT rows

            # ---- tile-0 key matmuls first: PE starts on the critical path
            # (taq needs sb_q only later, at its ACT op) ----
            ps_ak0 = ps.tile([LT, D], F32, tag="ak", name="ps_ak0")
            nc.tensor.matmul(ps_ak0[:, 0:512], sb_av[0:K1, 0:LT], sb_rhs[0:K1, 0:512], start=True, stop=True)
            nc.tensor.matmul(ps_ak0[:, 512:D], sb_av[0:K1, 0:LT], sb_rhs[0:K1, 512:D], start=True, stop=True)
            ps_vk0 = ps.tile([LT, D], F32, tag="vk", name="ps_vk0")
            nc.tensor.matmul(ps_vk0[:, 0:512], sb_av[VOFF:VOFF + K1, 0:LT], sb_rhs[VOFF:VOFF + K1, 0:512], start=True, stop=True)
            nc.tensor.matmul(ps_vk0[:, 512:D], sb_av[VOFF:VOFF + K1, 0:LT], sb_rhs[VOFF:VOFF + K1, 512:D], start=True, stop=True)

            # ---- qa/qv: per-l scalars via tiny (packed) matmuls ----
            ps_q = ps.tile([LT, 2 * NT], F32, tag="val")
            for i in range(NT):
                nc.tensor.matmul(
                    ps_q[:, 2 * i : 2 * i + 1],
                    sb_av[0:F, i * LT : (i + 1) * LT], sb_wq[0:F, :],
                    start=True, stop=True,
                )
                nc.tensor.matmul(
                    ps_q[:, 2 * i + 1 : 2 * i + 2],
                    sb_av[VOFF : VOFF + F, i * LT : (i + 1) * LT], sb_wq[VOFF : VOFF + F, :],
                    start=True, stop=True,
                )
            sb_q = consts.tile([LT, 2 * NT], F32, tag="sb_q")
            nc.vector.tensor_copy(out=sb_q[:], in_=ps_q[:])

            s_ta = consts.tile([LT, NT], F32, tag="s_ta")
            s_tv = consts.tile([LT, NT], F32, tag="s_tv")
            out_v_sb = consts.tile([LT, NT_HALF, D], F32, tag="out_v_sb")
            out_a_sb = consts.tile([LT, NT_HALF, D], F32, tag="out_a_sb")
            vvals, rss, rrs = [], [], []

            def emit_xa(j):
                """value-phase A-side: x_a2 matmuls + sin/cos argument prep
                (DVE frees the psum slot quickly)."""
                lsl = slice(j * LT, (j + 1) * LT)
                ps_xa = ps.tile([LT, D], F32, tag="val", name=f"ps_xa{j}")
                nc.tensor.matmul(ps_xa[:, 0:512], sb_av[A0:A1, lsl],
                                 sb_rhs[A0:A1, D : D + 512], start=True, stop=True)
                nc.tensor.matmul(ps_xa[:, 512:D], sb_av[A0:A1, lsl],
                                 sb_rhs[A0:A1, D + 512 : 2 * D], start=True, stop=True)
                rs = vals.tile([LT, D], F32, tag=f"rs{j}", name=f"rs{j}")
                nc.vector.add_range_wrap(out=rs[:], in_=ps_xa[:], shift=0.0, bound=PI, period=2 * PI)
                nax = vwork.tile([LT, D], F32, tag="nax", name=f"nax{j}")
                nc.vector.scalar_tensor_tensor(
                    out=nax[:], in0=rs[:], scalar=-1.0, in1=rs[:],
                    op0=ALU.mult, op1=ALU.min,
                )
                rr = vals.tile([LT, D], F32, tag=f"rr{j}", name=f"rr{j}")
                nc.vector.tensor_scalar(out=rr[:], in0=nax[:], scalar1=PIO2_HI,
                                        scalar2=None, op0=ALU.add)
                rss.append(rs)
                rrs.append(rr)

            def emit_xv(j):
                """value-phase V-side: x_v2 matmuls + vval tanh (ACT op is
                interleaved into the score-phase ACT stream; same table set)."""
                lsl = slice(j * LT, (j + 1) * LT)
                ps_xv = ps.tile([LT, D], F32, tag="val", name=f"ps_xv{j}")
                nc.tensor.matmul(ps_xv[:, 0:512], sb_av[V0:V1, lsl],
                                 sb_rhs[V0:V1, D : D + 512], start=True, stop=True)
                nc.tensor.matmul(ps_xv[:, 512:D], sb_av[V0:V1, lsl],
                                 sb_rhs[V0:V1, D + 512 : 2 * D], start=True, stop=True)
                vval = vals.tile([LT, D], F32, tag=f"vval{j}", name=f"vval{j}")
                nc.scalar.activation(out=vval[:], in_=ps_xv[:], func=AF.Tanh)
                vvals.append(vval)

            # ---- score phase over full L, with value work woven in ----
            for i in range(NT):
                lsl = slice(i * LT, (i + 1) * LT)
                # keypair psum [AKey | VKey]: bank0=a(512), bank1=a(256)+v(256),
                # bank2=v(512); issue order a1,v1,v2,a2 so the shared bank is
                # never written concurrently and A/V row-groups overlap.
                # split ak/vk psum tiles: AKey's tanh overlaps VKey's matmuls
                if i == 0:
                    ps_ak, ps_vk = ps_ak0, ps_vk0
                else:
                    ps_ak = ps.tile([LT, D], F32, tag="ak", name=f"ps_ak{i}")
                    nc.tensor.matmul(ps_ak[:, 0:512], sb_av[A0:A1, lsl], sb_rhs[A0:A1, 0:512], start=True, stop=True)
                    nc.tensor.matmul(ps_ak[:, 512:D], sb_av[A0:A1, lsl], sb_rhs[A0:A1, 512:D], start=True, stop=True)
                    ps_vk = ps.tile([LT, D], F32, tag="vk", name=f"ps_vk{i}")
                    nc.tensor.matmul(ps_vk[:, 0:512], sb_av[V0:V1, lsl], sb_rhs[V0:V1, 0:512], start=True, stop=True)
                    nc.tensor.matmul(ps_vk[:, 512:D], sb_av[V0:V1, lsl], sb_rhs[V0:V1, 512:D], start=True, stop=True)
                akey = keys.tile([LT, D], F32, tag="akey")
                nc.scalar.activation(out=akey[:], in_=ps_ak[:], func=AF.Tanh)
                taq = keys.tile([LT, D], F32, tag="taq")
                nc.scalar.activation(out=taq[:], in_=t_all[:, i, :], func=AF.Tanh,
                                     bias=sb_b[:, 0:1], scale=sb_q[:, 2 * i : 2 * i + 1])
                vkey = keys.tile([LT, D], F32, tag="vkey")
                nc.scalar.activation(out=vkey[:], in_=ps_vk[:], func=AF.Tanh)
                tvq = keys.tile([LT, D], F32, tag="tvq")
                nc.scalar.activation(out=tvq[:], in_=t_all[:, i, :], func=AF.Tanh,
                                     bias=sb_b[:, 1:2], scale=sb_q[:, 2 * i + 1 : 2 * i + 2])

                scr = keys.tile([LT, D], F32, tag="scr")
                nc.vector.scalar_tensor_tensor(
                    out=scr[:], in0=taq[:], scalar=1.0, in1=vkey[:],
                    op0=ALU.mult, op1=ALU.mult, accum_out=s_ta[:, i : i + 1],
                )
                scr2 = keys.tile([LT, D], F32, tag="scr2")
                nc.vector.scalar_tensor_tensor(
                    out=scr2[:], in0=tvq[:], scalar=1.0, in1=akey[:],
                    op0=ALU.mult, op1=ALU.mult, accum_out=s_tv[:, i : i + 1],
                )

                # weave value-phase work into the score stream
                if i % 2 == 1:
                    emit_xa(i // 2)
                elif i >= 2:
                    emit_xv(i // 2 - 1)
            emit_xv(3)

            # ---- softmax over all 1024 l's (no max subtraction; |s| < 40) ----
            e_ta = consts.tile([LT, NT], F32, tag="e_ta")
            e_tv = consts.tile([LT, NT], F32, tag="e_tv")
            rsum = consts.tile([LT, 2], F32, tag="rsum")
            nc.scalar.activation(out=e_ta[:], in_=s_ta[:], func=AF.Exp, accum_out=rsum[:, 0:1])
            exp_inst = nc.scalar.activation(out=e_tv[:], in_=s_tv[:], func=AF.Exp, accum_out=rsum[:, 1:2])
            zsum = consts.tile([LT, 2], F32, tag="zsum")
            nc.gpsimd.partition_all_reduce(zsum[:], rsum[:], channels=LT,
                                           reduce_op=bass_isa.ReduceOp.add)
            invzb = consts.tile([LT, 2], F32, tag="invzb")
            nc.vector.reciprocal(out=invzb[:], in_=zsum[:])
            ta_n = consts.tile([LT, NT_HALF], F32, tag="ta_n")
            nc.vector.tensor_scalar(out=ta_n[:], in0=e_ta[:, 0:NT_HALF],
                                    scalar1=invzb[:, 0:1], scalar2=None, op0=ALU.mult)
            tv_n = consts.tile([LT, NT_HALF], F32, tag="tv_n")
            nc.vector.tensor_scalar(out=tv_n[:], in0=e_tv[:, 0:NT_HALF],
                                    scalar1=invzb[:, 1:2], scalar2=None, op0=ALU.mult)

            # ---- value phase tail: scale vval, sin/cos, reciprocal, outputs ----
            for j in range(NT_HALF):
                nc.vector.tensor_scalar(out=out_v_sb[:, j, :], in0=vvals[j][:],
                                        scalar1=tv_n[:, j : j + 1], scalar2=None, op0=ALU.mult)
                if j == 1:
                    nc.sync.dma_start(out=ov_view[:, 0:2, :], in_=out_v_sb[:, 0:2, :])
            nc.sync.dma_start(out=ov_view[:, 2:4, :], in_=out_v_sb[:, 2:4, :])

            from concourse.tile import add_dep_helper
            for j in range(NT_HALF):
                sn = vwork.tile([LT, D], F32, tag="sn", bufs=4)
                i1 = nc.scalar.activation(out=sn[:], in_=rss[j][:], func=AF.Sin)
                cs = vwork.tile([LT, D], F32, tag="cs", bufs=4)
                i2 = nc.scalar.activation(out=cs[:], in_=rrs[j][:], func=AF.Sin, bias=sb_lo[:])
                # keep all Sin ops after the tanh/exp phase: one table switch
                add_dep_helper(i1.ins, exp_inst.ins, sync=False, reason="sin after exp (ACT table set)")
                add_dep_helper(i2.ins, exp_inst.ins, sync=False, reason="sin after exp (ACT table set)")
                rc = vwork.tile([LT, D], F32, tag="rc")
                nc.vector.reciprocal_approx_fast(out=rc[:], in_=cs[:])
                nc.vector.scalar_tensor_tensor(
                    out=out_a_sb[:, j, :], in0=sn[:], scalar=ta_n[:, j : j + 1], in1=rc[:],
                    op0=ALU.mult, op1=ALU.mult,
                )
                nc.sync.dma_start(out=oa_view[:, j : j + 1, :], in_=out_a_sb[:, j : j + 1, :])

    nc.finalize()
    _CACHE["nc"] = nc
    return nc


def _prep_in_maps(T, A, V, w_a, b_a, w_v, b_v,
                  W_aup1, b_aup1, W_aup2, b_aup2,
                  W_vup1, b_vup1, W_vup2, b_vup2):
    f32 = np.float32
    T = np.ascontiguousarray(np.asarray(T, f32))
    A = np.asarray(A, f32)
    V = np.asarray(V, f32)

    def aug_w(W, b):
        return np.concatenate([np.asarray(W, f32).T, np.asarray(b, f32)[None, :]], axis=0)

    rhs_pack = np.zeros((VOFF + K1, 2 * D), f32)
    rhs_pack[0:K1, 0:D] = aug_w(W_aup1, b_aup1)
    rhs_pack[0:K1, D : 2 * D] = aug_w(W_aup2, b_aup2)
    rhs_pack[VOFF : VOFF + K1, 0:D] = aug_w(W_vup1, b_vup1)
    rhs_pack[VOFF : VOFF + K1, D : 2 * D] = aug_w(W_vup2, b_vup2)

    w_q = np.zeros((VOFF + F, 1), f32)
    w_q[0:F, 0] = np.asarray(w_a, f32).reshape(F)
    w_q[VOFF : VOFF + F, 0] = np.asarray(w_v, f32).reshape(F)

    b_ab = np.empty((LT, 2), f32)
    b_ab[:, 0] = np.asarray(b_a, f32).reshape(())
    b_ab[:, 1] = np.asarray(b_v, f32).reshape(())

    ones = np.ones((1, L), f32)
    in_maps = []
    for c in range(NCORES):
        b, h = divmod(c, 2)
        rot = np.r_[np.arange(512 * h, L), np.arange(0, 512 * h)]
        av_pack = np.zeros((VOFF + K1, L), f32)
        av_pack[0:F] = A[b].T[:, rot]
        av_pack[F] = 1.0
        av_pack[VOFF : VOFF + F] = V[b].T[:, rot]
        av_pack[VOFF + F] = 1.0
        in_maps.append({
            "t_rot": np.ascontiguousarray(T[b][rot]),
            "av_pack": av_pack,
            "rhs_pack": rhs_pack,
            "w_q": w_q,
            "b_ab": b_ab,
        })
    return in_maps


def kernel(**inputs):
    from concourse.bass_utils import run_bass_kernel_spmd

    nc = _build()
    in_maps = _prep_in_maps(**inputs)
    res = run_bass_kernel_spmd(nc, in_maps, core_ids=list(range(NCORES)))

    out_a = np.empty((B, L, D), np.float32)
    out_v = np.empty((B, L, D), np.float32)
    for c in range(NCORES):
        b, h = divmod(c, 2)
        out_a[b, 512 * h : 512 * (h + 1)] = res.results[c]["out_a"]
        out_v[b, 512 * h : 512 * (h + 1)] = res.results[c]["out_v"]
    return out_a, out_v


# revision 29
# speedup vs baseline: 1.0077x; 1.0077x over previous
"""Trainium2 Bass kernel for nn_BEM_50002009260181.

Module (B=4, L=1024, D=768, F=32):
    AKey   = tanh(A @ W_aup1.T + b_aup1)          (B,L,D)
    AValue = tan (A @ W_aup2.T + b_aup2)          (B,L,D)
    VKey   = tanh(V @ W_vup1.T + b_vup1)          (B,L,D)
    VValue = tanh(V @ W_vup2.T + b_vup2)          (B,L,D)
    TAQ    = tanh(T * (A @ w_a.T) + b_a)          (B,L,D)
    TVQ    = tanh(T * (V @ w_v.T) + b_v)          (B,L,D)
    ta     = softmax_L(sum_d TAQ*VKey)            (B,L)
    tv     = softmax_L(sum_d TVQ*AKey)            (B,L)
    out    = (AValue * ta[...,None], VValue * tv[...,None])

Sharding: 8 cores = (batch b, L-half h).  Each core computes the full-L
scores for its batch (duplicated across the 2 cores of a batch, avoiding
any cross-core communication for the softmax) and the outputs for its own
L-half.  Inputs are rotated per-core so the own half is always tiles 0-3.

Layout: L on partitions (8 l-tiles of 128), D on the free dim.  The
Linear(32->768) weights ride as rhs of K=33 matmuls (bias folded in via a
ones-row in the lhsT).  A-side operands live on partitions 0-32 and V-side
on 64-96, so A/V matmul pairs land in disjoint PE row-groups and execute
concurrently.  TAQ/TVQ are a single ACT op each (per-partition scale=q,
bias=b).  Score reductions are fused mul+reduce (STT accum).  Softmax
skips max-subtraction (|scores| < 40 << 88).  tan = sin/cos with sin via
add_range_wrap into [-pi,pi] and a Cody-Waite cos exact near the poles.
"""

import numpy as np

B, L, D, F = 4, 1024, 768, 32
NCORES = 8
LT = 128          # l-tile size (partition dim)
NT = L // LT      # 8 l-tiles per batch
NT_HALF = NT // 2 # 4 own tiles
K1 = F + 1        # contraction with bias row
VOFF = 64         # partition offset of the V-side operands

PI = float(np.pi)
PIO2_HI = float(np.float32(np.pi / 2))
PIO2_LO = float(np.float64(np.pi / 2) - np.float64(np.float32(np.pi / 2)))

_CACHE = {}


def _build():
    if "nc" in _CACHE:
        return _CACHE["nc"]

    import concourse.bacc as bacc
    from concourse import bass_isa
    import concourse.tile as tile
    import concourse.mybir as mybir

    F32 = mybir.dt.float32
    AF = mybir.ActivationFunctionType
    ALU = mybir.AluOpType

    nc = bacc.Bacc()

    # ---- DRAM I/O (per-core shapes) ----
    d_t = nc.dram_tensor("t_rot", [L, D], F32, kind="ExternalInput")
    # av_pack rows: 0:33 = [A.T ; ones], 64:97 = [V.T ; ones]
    d_av = nc.dram_tensor("av_pack", [VOFF + K1, L], F32, kind="ExternalInput")
    # rhs_pack rows 0:33 = [Wa1.T|ba1 , Wa2.T|ba2], rows 64:97 = [Wv1.T|bv1 , Wv2.T|bv2]
    d_rhs = nc.dram_tensor("rhs_pack", [VOFF + K1, 2 * D], F32, kind="ExternalInput")
    d_wq = nc.dram_tensor("w_q", [VOFF + F, 1], F32, kind="ExternalInput")
    d_b = nc.dram_tensor("b_ab", [LT, 2], F32, kind="ExternalInput")
    d_oa = nc.dram_tensor("out_a", [L // 2, D], F32, kind="ExternalOutput")
    d_ov = nc.dram_tensor("out_v", [L // 2, D], F32, kind="ExternalOutput")

    t_view = d_t.rearrange("(n p) d -> p n d", p=LT)     # [128, 8, 768]
    oa_view = d_oa.rearrange("(n p) d -> p n d", p=LT)   # [128, 4, 768]
    ov_view = d_ov.rearrange("(n p) d -> p n d", p=LT)

    with tile.TileContext(nc) as tc:
        with (
            tc.tile_pool(name="consts", bufs=1) as consts,
            tc.tile_pool(name="keys", bufs=2) as keys,
            tc.tile_pool(name="vals", bufs=1) as vals,
            tc.tile_pool(name="vwork", bufs=3) as vwork,
            tc.tile_pool(name="ps", bufs=1, space="PSUM") as ps,
        ):
            # ---- inputs into SBUF ----
            # startup-critical DMAs first: tile-0's A-side matmul needs only
            # av rows 0:33 and rhs cols 0:512
            sb_av = consts.tile([VOFF + K1, L], F32, tag="sb_av")
            nc.sync.dma_start(out=sb_av[0:K1, :], in_=d_av[0:K1, :])
            sb_rhs = consts.tile([VOFF + K1, 2 * D], F32, tag="sb_rhs")
            nc.sync.dma_start(out=sb_rhs[0:K1, 0:512], in_=d_rhs[0:K1, 0:512])
            nc.sync.dma_start(out=sb_av[VOFF : VOFF + K1, :], in_=d_av[VOFF : VOFF + K1, :])
            nc.sync.dma_start(out=sb_rhs[VOFF : VOFF + K1, 0:512], in_=d_rhs[VOFF : VOFF + K1, 0:512])
            sb_wq = consts.tile([VOFF + F, 1], F32, tag="sb_wq")
            nc.sync.dma_start(out=sb_wq[:], in_=d_wq[:])
            nc.sync.dma_start(out=sb_rhs[0:K1, 512:D], in_=d_rhs[0:K1, 512:D])
            nc.sync.dma_start(out=sb_rhs[VOFF : VOFF + K1, 512:D], in_=d_rhs[VOFF : VOFF + K1, 512:D])
            sb_b = consts.tile([LT, 2], F32, tag="sb_b")
            nc.sync.dma_start(out=sb_b[:], in_=d_b[:])
            nc.sync.dma_start(out=sb_rhs[:, D : 2 * D], in_=d_rhs[:, D : 2 * D])
            t_all = consts.tile([LT, NT, D], F32, tag="t_all")
            nc.sync.dma_start(out=t_all[:, 0:1, :], in_=t_view[:, 0:1, :])
            nc.sync.dma_start(out=t_all[:, 1:4, :], in_=t_view[:, 1:4, :])
            nc.sync.dma_start(out=t_all[:, 4:NT, :], in_=t_view[:, 4:NT, :])

            sb_lo = consts.tile([LT, 1], F32, tag="sb_lo")
            nc.gpsimd.memset(sb_lo[:], PIO2_LO)
            # dummy first ACT op: pulls the tanh/exp table load to t=0 instead
            # of serializing it behind the first data-dependent activation
            warm = consts.tile([LT, 2], F32, tag="warm")
            nc.gpsimd.memset(warm[:], 0.0)
            nc.scalar.activation(out=warm[:, 1:2], in_=warm[:, 0:1], func=AF.Tanh)

            A0, A1 = 0, K1                  # a-side lhsT rows
            V0, V1 = VOFF, VOFF + K1        # v-side lhsT rows

            # ---- tile-0 key matmuls first: PE starts on the critical path
            # (taq needs sb_q only later, at its ACT op) ----
            ps_ak0 = ps.tile([LT, D], F32, tag="ak", name="ps_ak0")
            nc.tensor.matmul(ps_ak0[:, 0:512], sb_av[0:K1, 0:LT], sb_rhs[0:K1, 0:512], start=True, stop=True)
            nc.tensor.matmul(ps_ak0[:, 512:D], sb_av[0:K1, 0:LT], sb_rhs[0:K1, 512:D], start=True, stop=True)
            ps_vk0 = ps.tile([LT, D], F32, tag="vk", name="ps_vk0")
            nc.tensor.matmul(ps_vk0[:, 0:512], sb_av[VOFF:VOFF + K1, 0:LT], sb_rhs[VOFF:VOFF + K1, 0:512], start=True, stop=True)
            nc.tensor.matmul(ps_vk0[:, 512:D], sb_av[VOFF:VOFF + K1, 0:LT], sb_rhs[VOFF:VOFF + K1, 512:D], start=True, stop=True)

            # ---- qa/qv: per-l scalars via tiny (packed) matmuls ----
            ps_q = ps.tile([LT, 2 * NT], F32, tag="val")
            for i in range(NT):
                nc.tensor.matmul(
                    ps_q[:, 2 * i : 2 * i + 1],
                    sb_av[0:F, i * LT : (i + 1) * LT], sb_wq[0:F, :],
                    start=True, stop=True,
                )
                nc.tensor.matmul(
                    ps_q[:, 2 * i + 1 : 2 * i + 2],
                    sb_av[VOFF : VOFF + F, i * LT : (i + 1) * LT], sb_wq[VOFF : VOFF + F, :],
                    start=True, stop=True,
                )
            sb_q = consts.tile([LT, 2 * NT], F32, tag="sb_q")
            nc.vector.tensor_copy(out=sb_q[:], in_=ps_q[:])

            s_ta = consts.tile([LT, NT], F32, tag="s_ta")
            s_tv = consts.tile([LT, NT], F32, tag="s_tv")
            out_v_sb = consts.tile([LT, NT_HALF, D], F32, tag="out_v_sb")
            out_a_sb = consts.tile([LT, NT_HALF, D], F32, tag="out_a_sb")
            vvals, rss, rrs = [], [], []

            def emit_xa(j):
                """value-phase A-side: x_a2 matmuls + sin/cos argument prep
                (DVE frees the psum slot quickly)."""
                lsl = slice(j * LT, (j + 1) * LT)
                ps_xa = ps.tile([LT, D], F32, tag="val", name=f"ps_xa{j}")
                nc.tensor.matmul(ps_xa[:, 0:512], sb_av[A0:A1, lsl],
                                 sb_rhs[A0:A1, D : D + 512], start=True, stop=True)
                nc.tensor.matmul(ps_xa[:, 512:D], sb_av[A0:A1, lsl],
                                 sb_rhs[A0:A1, D + 512 : 2 * D], start=True, stop=True)
                rs = vals.tile([LT, D], F32, tag=f"rs{j}", name=f"rs{j}")
                nc.vector.add_range_wrap(out=rs[:], in_=ps_xa[:], shift=0.0, bound=PI, period=2 * PI)
                nax = vwork.tile([LT, D], F32, tag="nax", name=f"nax{j}")
                nc.vector.scalar_tensor_tensor(
                    out=nax[:], in0=rs[:], scalar=-1.0, in1=rs[:],
                    op0=ALU.mult, op1=ALU.min,
                )
                rr = vals.tile([LT, D], F32, tag=f"rr{j}", name=f"rr{j}")
                nc.vector.tensor_scalar(out=rr[:], in0=nax[:], scalar1=PIO2_HI,
                                        scalar2=None, op0=ALU.add)
                rss.append(rs)
                rrs.append(rr)

            def emit_xv(j):
                """value-phase V-side: x_v2 matmuls + vval tanh (ACT op is
                interleaved into the score-phase ACT stream; same table set)."""
                lsl = slice(j * LT, (j + 1) * LT)
                ps_xv = ps.tile([LT, D], F32, tag="val", name=f"ps_xv{j}")
                nc.tensor.matmul(ps_xv[:, 0:512], sb_av[V0:V1, lsl],
                                 sb_rhs[V0:V1, D : D + 512], start=True, stop=True)
                nc.tensor.matmul(ps_xv[:, 512:D], sb_av[V0:V1, lsl],
                                 sb_rhs[V0:V1, D + 512 : 2 * D], start=True, stop=True)
                vval = vals.tile([LT, D], F32, tag=f"vval{j}", name=f"vval{j}")
                nc.scalar.activation(out=vval[:], in_=ps_xv[:], func=AF.Tanh)
                vvals.append(vval)

            # ---- score phase over full L, with value work woven in ----
            for i in range(NT):
                lsl = slice(i * LT, (i + 1) * LT)
                # keypair psum [AKey | VKey]: bank0=a(512), bank1=a(256)+v(256),
                # bank2=v(512); issue order a1,v1,v2,a2 so the shared bank is
                # never written concurrently and A/V row-groups overlap.
                # split ak/vk psum tiles: AKey's tanh overlaps VKey's matmuls
                if i == 0:
                    ps_ak, ps_vk = ps_ak0, ps_vk0
                else:
                    ps_ak = ps.tile([LT, D], F32, tag="ak", name=f"ps_ak{i}")
                    nc.tensor.matmul(ps_ak[:, 0:512], sb_av[A0:A1, lsl], sb_rhs[A0:A1, 0:512], start=True, stop=True)
                    nc.tensor.matmul(ps_ak[:, 512:D], sb_av[A0:A1, lsl], sb_rhs[A0:A1, 512:D], start=True, stop=True)
                    ps_vk = ps.tile([LT, D], F32, tag="vk", name=f"ps_vk{i}")
                    nc.tensor.matmul(ps_vk[:, 0:512], sb_av[V0:V1, lsl], sb_rhs[V0:V1, 0:512], start=True, stop=True)
                    nc.tensor.matmul(ps_vk[:, 512:D], sb_av[V0:V1, lsl], sb_rhs[V0:V1, 512:D], start=True, stop=True)
                akey = keys.tile([LT, D], F32, tag="akey")
                nc.scalar.activation(out=akey[:], in_=ps_ak[:], func=AF.Tanh)
                taq = keys.tile([LT, D], F32, tag="taq")
                nc.scalar.activation(out=taq[:], in_=t_all[:, i, :], func=AF.Tanh,
                                     bias=sb_b[:, 0:1], scale=sb_q[:, 2 * i : 2 * i + 1])
                vkey = keys.tile([LT, D], F32, tag="vkey")
                nc.scalar.activation(out=vkey[:], in_=ps_vk[:], func=AF.Tanh)
                tvq = keys.tile([LT, D], F32, tag="tvq")
                nc.scalar.activation(out=tvq[:], in_=t_all[:, i, :], func=AF.Tanh,
                                     bias=sb_b[:, 1:2], scale=sb_q[:, 2 * i + 1 : 2 * i + 2])

                scr = keys.tile([LT, D], F32, tag="scr")
                nc.vector.scalar_tensor_tensor(
                    out=scr[:], in0=taq[:], scalar=1.0, in1=vkey[:],
                    op0=ALU.mult, op1=ALU.mult, accum_out=s_ta[:, i : i + 1],
                )
                scr2 = keys.tile([LT, D], F32, tag="scr2")
                nc.vector.scalar_tensor_tensor(
                    out=scr2[:], in0=tvq[:], scalar=1.0, in1=akey[:],
                    op0=ALU.mult, op1=ALU.mult, accum_out=s_tv[:, i : i + 1],
                )

                # weave value-phase work into the score stream
                if i % 2 == 1:
                    emit_xa(i // 2)
                elif i >= 2:
                    emit_xv(i // 2 - 1)
            emit_xv(3)

            # ---- softmax over all 1024 l's (no max subtraction; |s| < 40) ----
            e_ta = consts.tile([LT, NT], F32, tag="e_ta")
            e_tv = consts.tile([LT, NT], F32, tag="e_tv")
            rsum = consts.tile([LT, 2], F32, tag="rsum")
            nc.scalar.activation(out=e_ta[:], in_=s_ta[:], func=AF.Exp, accum_out=rsum[:, 0:1])
            exp_inst = nc.scalar.activation(out=e_tv[:], in_=s_tv[:], func=AF.Exp, accum_out=rsum[:, 1:2])
            zsum = consts.tile([LT, 2], F32, tag="zsum")
            nc.gpsimd.partition_all_reduce(zsum[:], rsum[:], channels=LT,
                                           reduce_op=bass_isa.ReduceOp.add)
            invzb = consts.tile([LT, 2], F32, tag="invzb")
            nc.vector.reciprocal(out=invzb[:], in_=zsum[:])
            ta_n = consts.tile([LT, NT_HALF], F32, tag="ta_n")
            nc.vector.tensor_scalar(out=ta_n[:], in0=e_ta[:, 0:NT_HALF],
                                    scalar1=invzb[:, 0:1], scalar2=None, op0=ALU.mult)
            tv_n = consts.tile([LT, NT_HALF], F32, tag="tv_n")
            nc.vector.tensor_scalar(out=tv_n[:], in0=e_tv[:, 0:NT_HALF],
                                    scalar1=invzb[:, 1:2], scalar2=None, op0=ALU.mult)

            # ---- value phase tail: scale vval, sin/cos, reciprocal, outputs ----
            for j in range(NT_HALF):
                nc.vector.tensor_scalar(out=out_v_sb[:, j, :], in0=vvals[j][:],
                                        scalar1=tv_n[:, j : j + 1], scalar2=None, op0=ALU.mult)
                if j == 1:
                    nc.sync.dma_start(out=ov_view[:, 0:2, :], in_=out_v_sb[:, 0:2, :])
            nc.sync.dma_start(out=ov_view[:, 2:4, :], in_=out_v_sb[:, 2:4, :])

            from concourse.tile import add_dep_helper
            for j in range(NT_HALF):
                sn = vwork.tile([LT, D], F32, tag="sn", bufs=4)
                i1 = nc.scalar.activation(out=sn[:], in_=rss[j][:], func=AF.Sin)
                cs = vwork.tile([LT, D], F32, tag="cs", bufs=4)
                i2 = nc.scalar.activation(out=cs[:], in_=rrs[j][:], func=AF.Sin, bias=sb_lo[:])
                # keep all Sin ops after the tanh/exp phase: one table switch
                add_dep_helper(i1.ins, exp_inst.ins, sync=False, reason="sin after exp (ACT table set)")
                add_dep_helper(i2.ins, exp_inst.ins, sync=False, reason="sin after exp (ACT table set)")
                rc = vwork.tile([LT, D], F32, tag="rc")
                nc.vector.reciprocal_approx_fast(out=rc[:], in_=cs[:])
                nc.vector.scalar_tensor_tensor(
                    out=out_a_sb[:, j, :], in0=sn[:], scalar=ta_n[:, j : j + 1], in1=rc[:],
                    op0=ALU.mult, op1=ALU.mult,
                )
                nc.sync.dma_start(out=oa_view[:, j : j + 1, :], in_=out_a_sb[:, j : j + 1, :])

    nc.finalize()
    _CACHE["nc"] = nc
    return nc


def _prep_in_maps(T, A, V, w_a, b_a, w_v, b_v,
                  W_aup1, b_aup1, W_aup2, b_aup2,
                  W_vup1, b_vup1, W_vup2, b_vup2):
    f32 = np.float32
    T = np.ascontiguousarray(np.asarray(T, f32))
    A = np.asarray(A, f32)
    V = np.asarray(V, f32)

    def aug_w(W, b):
        return np.concatenate([np.asarray(W, f32).T, np.asarray(b, f32)[None, :]], axis=0)

    rhs_pack = np.zeros((VOFF + K1, 2 * D), f32)
    rhs_pack[0:K1, 0:D] = aug_w(W_aup1, b_aup1)
    rhs_pack[0:K1, D : 2 * D] = aug_w(W_aup2, b_aup2)
    rhs_pack[VOFF : VOFF + K1, 0:D] = aug_w(W_vup1, b_vup1)
    rhs_pack[VOFF : VOFF + K1, D : 2 * D] = aug_w(W_vup2, b_vup2)

    w_q = np.zeros((VOFF + F, 1), f32)
    w_q[0:F, 0] = np.asarray(w_a, f32).reshape(F)
    w_q[VOFF : VOFF + F, 0] = np.asarray(w_v, f32).reshape(F)

    b_ab = np.empty((LT, 2), f32)
    b_ab[:, 0] = np.asarray(b_a, f32).reshape(())
    b_ab[:, 1] = np.asarray(b_v, f32).reshape(())

    ones = np.ones((1, L), f32)
    in_maps = []
    for c in range(NCORES):
        b, h = divmod(c, 2)
        rot = np.r_[np.arange(512 * h, L), np.arange(0, 512 * h)]
        av_pack = np.zeros((VOFF + K1, L), f32)
        av_pack[0:F] = A[b].T[:, rot]
        av_pack[F] = 1.0
        av_pack[VOFF : VOFF + F] = V[b].T[:, rot]
        av_pack[VOFF + F] = 1.0
        in_maps.append({
            "t_rot": np.ascontiguousarray(T[b][rot]),
            "av_pack": av_pack,
            "rhs_pack": rhs_pack,
            "w_q": w_q,
            "b_ab": b_ab,
        })
    return in_maps


def kernel(**inputs):
    from concourse.bass_utils import run_bass_kernel_spmd

    nc = _build()
    in_maps = _prep_in_maps(**inputs)
    res = run_bass_kernel_spmd(nc, in_maps, core_ids=list(range(NCORES)))

    out_a = np.empty((B, L, D), np.float32)
    out_v = np.empty((B, L, D), np.float32)
    for c in range(NCORES):
        b, h = divmod(c, 2)
        out_a[b, 512 * h : 512 * (h + 1)] = res.results[c]["out_a"]
        out_v[b, 512 * h : 512 * (h + 1)] = res.results[c]["out_v"]
    return out_a, out_v


# revision 30
# speedup vs baseline: 1.0187x; 1.0109x over previous
"""Trainium2 Bass kernel for nn_BEM_50002009260181.

Module (B=4, L=1024, D=768, F=32):
    AKey   = tanh(A @ W_aup1.T + b_aup1)          (B,L,D)
    AValue = tan (A @ W_aup2.T + b_aup2)          (B,L,D)
    VKey   = tanh(V @ W_vup1.T + b_vup1)          (B,L,D)
    VValue = tanh(V @ W_vup2.T + b_vup2)          (B,L,D)
    TAQ    = tanh(T * (A @ w_a.T) + b_a)          (B,L,D)
    TVQ    = tanh(T * (V @ w_v.T) + b_v)          (B,L,D)
    ta     = softmax_L(sum_d TAQ*VKey)            (B,L)
    tv     = softmax_L(sum_d TVQ*AKey)            (B,L)
    out    = (AValue * ta[...,None], VValue * tv[...,None])

Sharding: 8 cores = (batch b, L-half h).  Each core computes the full-L
scores for its batch (duplicated across the 2 cores of a batch, avoiding
any cross-core communication for the softmax) and the outputs for its own
L-half.  Inputs are rotated per-core so the own half is always tiles 0-3.

Layout: L on partitions (8 l-tiles of 128), D on the free dim.  The
Linear(32->768) weights ride as rhs of K=33 matmuls (bias folded in via a
ones-row in the lhsT).  A-side operands live on partitions 0-32 and V-side
on 64-96, so A/V matmul pairs land in disjoint PE row-groups and execute
concurrently.  TAQ/TVQ are a single ACT op each (per-partition scale=q,
bias=b).  Score reductions are fused mul+reduce (STT accum).  Softmax
skips max-subtraction (|scores| < 40 << 88).  tan = sin/cos with sin via
add_range_wrap into [-pi,pi] and a Cody-Waite cos exact near the poles.
"""

import numpy as np

B, L, D, F = 4, 1024, 768, 32
NCORES = 8
LT = 128          # l-tile size (partition dim)
NT = L // LT      # 8 l-tiles per batch
NT_HALF = NT // 2 # 4 own tiles
K1 = F + 1        # contraction with bias row
VOFF = 64         # partition offset of the V-side operands

PI = float(np.pi)
PIO2_HI = float(np.float32(np.pi / 2))
PIO2_LO = float(np.float64(np.pi / 2) - np.float64(np.float32(np.pi / 2)))

_CACHE = {}


def _build():
    if "nc" in _CACHE:
        return _CACHE["nc"]

    import concourse.bacc as bacc
    from concourse import bass_isa
    import concourse.tile as tile
    import concourse.mybir as mybir

    F32 = mybir.dt.float32
    AF = mybir.ActivationFunctionType
    ALU = mybir.AluOpType

    nc = bacc.Bacc()

    # ---- DRAM I/O (per-core shapes) ----
    d_t = nc.dram_tensor("t_rot", [L, D], F32, kind="ExternalInput")
    # av_pack rows: 0:33 = [A.T ; ones], 64:97 = [V.T ; ones]
    d_av = nc.dram_tensor("av_pack", [VOFF + K1, L], F32, kind="ExternalInput")
    # rhs_pack rows 0:33 = [Wa1.T|ba1 , Wa2.T|ba2], rows 64:97 = [Wv1.T|bv1 , Wv2.T|bv2]
    d_rhs = nc.dram_tensor("rhs_pack", [VOFF + K1, 2 * D], F32, kind="ExternalInput")
    d_wq = nc.dram_tensor("w_q", [VOFF + F, 1], F32, kind="ExternalInput")
    d_b = nc.dram_tensor("b_ab", [LT, 2], F32, kind="ExternalInput")
    d_oa = nc.dram_tensor("out_a", [L // 2, D], F32, kind="ExternalOutput")
    d_ov = nc.dram_tensor("out_v", [L // 2, D], F32, kind="ExternalOutput")

    t_view = d_t.rearrange("(n p) d -> p n d", p=LT)     # [128, 8, 768]
    oa_view = d_oa.rearrange("(n p) d -> p n d", p=LT)   # [128, 4, 768]
    ov_view = d_ov.rearrange("(n p) d -> p n d", p=LT)

    with tile.TileContext(nc) as tc:
        with (
            tc.tile_pool(name="consts", bufs=1) as consts,
            tc.tile_pool(name="keys", bufs=2) as keys,
            tc.tile_pool(name="vals", bufs=1) as vals,
            tc.tile_pool(name="vwork", bufs=3) as vwork,
            tc.tile_pool(name="ps", bufs=1, space="PSUM") as ps,
        ):
            # ---- inputs into SBUF ----
            # startup-critical DMAs first: tile-0's A-side matmul needs only
            # av rows 0:33 and rhs cols 0:512
            sb_av = consts.tile([VOFF + K1, L], F32, tag="sb_av")
            nc.sync.dma_start(out=sb_av[0:K1, :], in_=d_av[0:K1, :])
            sb_rhs = consts.tile([VOFF + K1, 2 * D], F32, tag="sb_rhs")
            nc.sync.dma_start(out=sb_rhs[0:K1, 0:512], in_=d_rhs[0:K1, 0:512])
            nc.sync.dma_start(out=sb_av[VOFF : VOFF + K1, :], in_=d_av[VOFF : VOFF + K1, :])
            nc.sync.dma_start(out=sb_rhs[VOFF : VOFF + K1, 0:512], in_=d_rhs[VOFF : VOFF + K1, 0:512])
            sb_wq = consts.tile([VOFF + F, 1], F32, tag="sb_wq")
            nc.sync.dma_start(out=sb_wq[:], in_=d_wq[:])
            nc.sync.dma_start(out=sb_rhs[0:K1, 512:D], in_=d_rhs[0:K1, 512:D])
            nc.sync.dma_start(out=sb_rhs[VOFF : VOFF + K1, 512:D], in_=d_rhs[VOFF : VOFF + K1, 512:D])
            sb_b = consts.tile([LT, 2], F32, tag="sb_b")
            nc.sync.dma_start(out=sb_b[:], in_=d_b[:])
            nc.sync.dma_start(out=sb_rhs[:, D : 2 * D], in_=d_rhs[:, D : 2 * D])
            t_all = consts.tile([LT, NT, D], F32, tag="t_all")
            nc.sync.dma_start(out=t_all[:, 0:1, :], in_=t_view[:, 0:1, :])
            nc.sync.dma_start(out=t_all[:, 1:4, :], in_=t_view[:, 1:4, :])
            nc.sync.dma_start(out=t_all[:, 4:NT, :], in_=t_view[:, 4:NT, :])

            sb_lo = consts.tile([LT, 1], F32, tag="sb_lo")
            nc.gpsimd.memset(sb_lo[:], PIO2_LO)
            # dummy first ACT op: pulls the tanh/exp table load to t=0 instead
            # of serializing it behind the first data-dependent activation
            warm = consts.tile([LT, 2], F32, tag="warm")
            nc.gpsimd.memset(warm[:], 0.0)
            nc.scalar.activation(out=warm[:, 1:2], in_=warm[:, 0:1], func=AF.Tanh)

            A0, A1 = 0, K1                  # a-side lhsT rows
            V0, V1 = VOFF, VOFF + K1        # v-side lhsT rows

            # ---- PE warmup: dependency-free dummy matmuls keep PE busy from
            # t=0 so the first real matmuls run at the warm rate (and warm the
            # HAM clock gate on real hardware) ----
            dmy = consts.tile([F, 64], F32, tag="dmy")
            nc.gpsimd.memset(dmy[:], 0.0)
            ps_d = ps.tile([64, 64], F32, tag="val", name="ps_d")
            for _k in range(20):
                nc.tensor.matmul(ps_d[:], dmy[:, 0:64], dmy[:, 0:64], start=True, stop=True)

            # ---- tile-0 key matmuls first: PE starts on the critical path
            # (taq needs sb_q only later, at its ACT op) ----
            ps_ak0 = ps.tile([LT, D], F32, tag="ak", name="ps_ak0")
            nc.tensor.matmul(ps_ak0[:, 0:512], sb_av[0:K1, 0:LT], sb_rhs[0:K1, 0:512], start=True, stop=True)
            nc.tensor.matmul(ps_ak0[:, 512:D], sb_av[0:K1, 0:LT], sb_rhs[0:K1, 512:D], start=True, stop=True)
            ps_vk0 = ps.tile([LT, D], F32, tag="vk", name="ps_vk0")
            nc.tensor.matmul(ps_vk0[:, 0:512], sb_av[VOFF:VOFF + K1, 0:LT], sb_rhs[VOFF:VOFF + K1, 0:512], start=True, stop=True)
            nc.tensor.matmul(ps_vk0[:, 512:D], sb_av[VOFF:VOFF + K1, 0:LT], sb_rhs[VOFF:VOFF + K1, 512:D], start=True, stop=True)

            # ---- qa/qv: per-l scalars via tiny (packed) matmuls ----
            ps_q = ps.tile([LT, 2 * NT], F32, tag="val")
            for i in range(NT):
                nc.tensor.matmul(
                    ps_q[:, 2 * i : 2 * i + 1],
                    sb_av[0:F, i * LT : (i + 1) * LT], sb_wq[0:F, :],
                    start=True, stop=True,
                )
                nc.tensor.matmul(
                    ps_q[:, 2 * i + 1 : 2 * i + 2],
                    sb_av[VOFF : VOFF + F, i * LT : (i + 1) * LT], sb_wq[VOFF : VOFF + F, :],
                    start=True, stop=True,
                )
            sb_q = consts.tile([LT, 2 * NT], F32, tag="sb_q")
            nc.vector.tensor_copy(out=sb_q[:], in_=ps_q[:])

            s_ta = consts.tile([LT, NT], F32, tag="s_ta")
            s_tv = consts.tile([LT, NT], F32, tag="s_tv")
            out_v_sb = consts.tile([LT, NT_HALF, D], F32, tag="out_v_sb")
            out_a_sb = consts.tile([LT, NT_HALF, D], F32, tag="out_a_sb")
            vvals, rss, rrs = [], [], []

            def emit_xa(j):
                """value-phase A-side: x_a2 matmuls + sin/cos argument prep
                (DVE frees the psum slot quickly)."""
                lsl = slice(j * LT, (j + 1) * LT)
                ps_xa = ps.tile([LT, D], F32, tag="val", name=f"ps_xa{j}")
                nc.tensor.matmul(ps_xa[:, 0:512], sb_av[A0:A1, lsl],
                                 sb_rhs[A0:A1, D : D + 512], start=True, stop=True)
                nc.tensor.matmul(ps_xa[:, 512:D], sb_av[A0:A1, lsl],
                                 sb_rhs[A0:A1, D + 512 : 2 * D], start=True, stop=True)
                rs = vals.tile([LT, D], F32, tag=f"rs{j}", name=f"rs{j}")
                nc.vector.add_range_wrap(out=rs[:], in_=ps_xa[:], shift=0.0, bound=PI, period=2 * PI)
                nax = vwork.tile([LT, D], F32, tag="nax", name=f"nax{j}")
                nc.vector.scalar_tensor_tensor(
                    out=nax[:], in0=rs[:], scalar=-1.0, in1=rs[:],
                    op0=ALU.mult, op1=ALU.min,
                )
                rr = vals.tile([LT, D], F32, tag=f"rr{j}", name=f"rr{j}")
                nc.vector.tensor_scalar(out=rr[:], in0=nax[:], scalar1=PIO2_HI,
                                        scalar2=None, op0=ALU.add)
                rss.append(rs)
                rrs.append(rr)

            def emit_xv(j):
                """value-phase V-side: x_v2 matmuls + vval tanh (ACT op is
                interleaved into the score-phase ACT stream; same table set)."""
                lsl = slice(j * LT, (j + 1) * LT)
                ps_xv = ps.tile([LT, D], F32, tag="val", name=f"ps_xv{j}")
                nc.tensor.matmul(ps_xv[:, 0:512], sb_av[V0:V1, lsl],
                                 sb_rhs[V0:V1, D : D + 512], start=True, stop=True)
                nc.tensor.matmul(ps_xv[:, 512:D], sb_av[V0:V1, lsl],
                                 sb_rhs[V0:V1, D + 512 : 2 * D], start=True, stop=True)
                vval = vals.tile([LT, D], F32, tag=f"vval{j}", name=f"vval{j}")
                nc.scalar.activation(out=vval[:], in_=ps_xv[:], func=AF.Tanh)
                vvals.append(vval)

            # ---- score phase over full L, with value work woven in ----
            for i in range(NT):
                lsl = slice(i * LT, (i + 1) * LT)
                # keypair psum [AKey | VKey]: bank0=a(512), bank1=a(256)+v(256),
                # bank2=v(512); issue order a1,v1,v2,a2 so the shared bank is
                # never written concurrently and A/V row-groups overlap.
                # split ak/vk psum tiles: AKey's tanh overlaps VKey's matmuls
                if i == 0:
                    ps_ak, ps_vk = ps_ak0, ps_vk0
                else:
                    ps_ak = ps.tile([LT, D], F32, tag="ak", name=f"ps_ak{i}")
                    nc.tensor.matmul(ps_ak[:, 0:512], sb_av[A0:A1, lsl], sb_rhs[A0:A1, 0:512], start=True, stop=True)
                    nc.tensor.matmul(ps_ak[:, 512:D], sb_av[A0:A1, lsl], sb_rhs[A0:A1, 512:D], start=True, stop=True)
                    ps_vk = ps.tile([LT, D], F32, tag="vk", name=f"ps_vk{i}")
                    nc.tensor.matmul(ps_vk[:, 0:512], sb_av[V0:V1, lsl], sb_rhs[V0:V1, 0:512], start=True, stop=True)
                    nc.tensor.matmul(ps_vk[:, 512:D], sb_av[V0:V1, lsl], sb_rhs[V0:V1, 512:D], start=True, stop=True)
                akey = keys.tile([LT, D], F32, tag="akey")
                nc.scalar.activation(out=akey[:], in_=ps_ak[:], func=AF.Tanh)
                taq = keys.tile([LT, D], F32, tag="taq")
                nc.scalar.activation(out=taq[:], in_=t_all[:, i, :], func=AF.Tanh,
                                     bias=sb_b[:, 0:1], scale=sb_q[:, 2 * i : 2 * i + 1])
                vkey = keys.tile([LT, D], F32, tag="vkey")
                nc.scalar.activation(out=vkey[:], in_=ps_vk[:], func=AF.Tanh)
                tvq = keys.tile([LT, D], F32, tag="tvq")
                nc.scalar.activation(out=tvq[:], in_=t_all[:, i, :], func=AF.Tanh,
                                     bias=sb_b[:, 1:2], scale=sb_q[:, 2 * i + 1 : 2 * i + 2])

                scr = keys.tile([LT, D], F32, tag="scr")
                nc.vector.scalar_tensor_tensor(
                    out=scr[:], in0=taq[:], scalar=1.0, in1=vkey[:],
                    op0=ALU.mult, op1=ALU.mult, accum_out=s_ta[:, i : i + 1],
                )
                scr2 = keys.tile([LT, D], F32, tag="scr2")
                nc.vector.scalar_tensor_tensor(
                    out=scr2[:], in0=tvq[:], scalar=1.0, in1=akey[:],
                    op0=ALU.mult, op1=ALU.mult, accum_out=s_tv[:, i : i + 1],
                )

                # weave value-phase work into the score stream
                if i % 2 == 1:
                    emit_xa(i // 2)
                elif i >= 2:
                    emit_xv(i // 2 - 1)
            emit_xv(3)

            # ---- softmax over all 1024 l's (no max subtraction; |s| < 40) ----
            e_ta = consts.tile([LT, NT], F32, tag="e_ta")
            e_tv = consts.tile([LT, NT], F32, tag="e_tv")
            rsum = consts.tile([LT, 2], F32, tag="rsum")
            nc.scalar.activation(out=e_ta[:], in_=s_ta[:], func=AF.Exp, accum_out=rsum[:, 0:1])
            exp_inst = nc.scalar.activation(out=e_tv[:], in_=s_tv[:], func=AF.Exp, accum_out=rsum[:, 1:2])
            zsum = consts.tile([LT, 2], F32, tag="zsum")
            nc.gpsimd.partition_all_reduce(zsum[:], rsum[:], channels=LT,
                                           reduce_op=bass_isa.ReduceOp.add)
            invzb = consts.tile([LT, 2], F32, tag="invzb")
            nc.vector.reciprocal(out=invzb[:], in_=zsum[:])
            ta_n = consts.tile([LT, NT_HALF], F32, tag="ta_n")
            nc.vector.tensor_scalar(out=ta_n[:], in0=e_ta[:, 0:NT_HALF],
                                    scalar1=invzb[:, 0:1], scalar2=None, op0=ALU.mult)
            tv_n = consts.tile([LT, NT_HALF], F32, tag="tv_n")
            nc.vector.tensor_scalar(out=tv_n[:], in0=e_tv[:, 0:NT_HALF],
                                    scalar1=invzb[:, 1:2], scalar2=None, op0=ALU.mult)

            # ---- value phase tail: scale vval, sin/cos, reciprocal, outputs ----
            for j in range(NT_HALF):
                nc.vector.tensor_scalar(out=out_v_sb[:, j, :], in0=vvals[j][:],
                                        scalar1=tv_n[:, j : j + 1], scalar2=None, op0=ALU.mult)
                if j == 1:
                    nc.sync.dma_start(out=ov_view[:, 0:2, :], in_=out_v_sb[:, 0:2, :])
            nc.sync.dma_start(out=ov_view[:, 2:4, :], in_=out_v_sb[:, 2:4, :])

            from concourse.tile import add_dep_helper
            for j in range(NT_HALF):
                sn = vwork.tile([LT, D], F32, tag="sn", bufs=4)
                i1 = nc.scalar.activation(out=sn[:], in_=rss[j][:], func=AF.Sin)
                cs = vwork.tile([LT, D], F32, tag="cs", bufs=4)
                i2 = nc.scalar.activation(out=cs[:], in_=rrs[j][:], func=AF.Sin, bias=sb_lo[:])
                # keep all Sin ops after the tanh/exp phase: one table switch
                add_dep_helper(i1.ins, exp_inst.ins, sync=False, reason="sin after exp (ACT table set)")
                add_dep_helper(i2.ins, exp_inst.ins, sync=False, reason="sin after exp (ACT table set)")
                rc = vwork.tile([LT, D], F32, tag="rc")
                nc.vector.reciprocal_approx_fast(out=rc[:], in_=cs[:])
                nc.vector.scalar_tensor_tensor(
                    out=out_a_sb[:, j, :], in0=sn[:], scalar=ta_n[:, j : j + 1], in1=rc[:],
                    op0=ALU.mult, op1=ALU.mult,
                )
                nc.sync.dma_start(out=oa_view[:, j : j + 1, :], in_=out_a_sb[:, j : j + 1, :])

    nc.finalize()
    _CACHE["nc"] = nc
    return nc


def _prep_in_maps(T, A, V, w_a, b_a, w_v, b_v,
                  W_aup1, b_aup1, W_aup2, b_aup2,
                  W_vup1, b_vup1, W_vup2, b_vup2):
    f32 = np.float32
    T = np.ascontiguousarray(np.asarray(T, f32))
    A = np.asarray(A, f32)
    V = np.asarray(V, f32)

    def aug_w(W, b):
        return np.concatenate([np.asarray(W, f32).T, np.asarray(b, f32)[None, :]], axis=0)

    rhs_pack = np.zeros((VOFF + K1, 2 * D), f32)
    rhs_pack[0:K1, 0:D] = aug_w(W_aup1, b_aup1)
    rhs_pack[0:K1, D : 2 * D] = aug_w(W_aup2, b_aup2)
    rhs_pack[VOFF : VOFF + K1, 0:D] = aug_w(W_vup1, b_vup1)
    rhs_pack[VOFF : VOFF + K1, D : 2 * D] = aug_w(W_vup2, b_vup2)

    w_q = np.zeros((VOFF + F, 1), f32)
    w_q[0:F, 0] = np.asarray(w_a, f32).reshape(F)
    w_q[VOFF : VOFF + F, 0] = np.asarray(w_v, f32).reshape(F)

    b_ab = np.empty((LT, 2), f32)
    b_ab[:, 0] = np.asarray(b_a, f32).reshape(())
    b_ab[:, 1] = np.asarray(b_v, f32).reshape(())

    ones = np.ones((1, L), f32)
    in_maps = []
    for c in range(NCORES):
        b, h = divmod(c, 2)
        rot = np.r_[np.arange(512 * h, L), np.arange(0, 512 * h)]
        av_pack = np.zeros((VOFF + K1, L), f32)
        av_pack[0:F] = A[b].T[:, rot]
        av_pack[F] = 1.0
        av_pack[VOFF : VOFF + F] = V[b].T[:, rot]
        av_pack[VOFF + F] = 1.0
        in_maps.append({
            "t_rot": np.ascontiguousarray(T[b][rot]),
            "av_pack": av_pack,
            "rhs_pack": rhs_pack,
            "w_q": w_q,
            "b_ab": b_ab,
        })
    return in_maps


def kernel(**inputs):
    from concourse.bass_utils import run_bass_kernel_spmd

    nc = _build()
    in_maps = _prep_in_maps(**inputs)
    res = run_bass_kernel_spmd(nc, in_maps, core_ids=list(range(NCORES)))

    out_a = np.empty((B, L, D), np.float32)
    out_v = np.empty((B, L, D), np.float32)
    for c in range(NCORES):
        b, h = divmod(c, 2)
        out_a[b, 512 * h : 512 * (h + 1)] = res.results[c]["out_a"]
        out_v[b, 512 * h : 512 * (h + 1)] = res.results[c]["out_v"]
    return out_a, out_v


# revision 31
# speedup vs baseline: 1.0262x; 1.0074x over previous
"""Trainium2 Bass kernel for nn_BEM_50002009260181.

Module (B=4, L=1024, D=768, F=32):
    AKey   = tanh(A @ W_aup1.T + b_aup1)          (B,L,D)
    AValue = tan (A @ W_aup2.T + b_aup2)          (B,L,D)
    VKey   = tanh(V @ W_vup1.T + b_vup1)          (B,L,D)
    VValue = tanh(V @ W_vup2.T + b_vup2)          (B,L,D)
    TAQ    = tanh(T * (A @ w_a.T) + b_a)          (B,L,D)
    TVQ    = tanh(T * (V @ w_v.T) + b_v)          (B,L,D)
    ta     = softmax_L(sum_d TAQ*VKey)            (B,L)
    tv     = softmax_L(sum_d TVQ*AKey)            (B,L)
    out    = (AValue * ta[...,None], VValue * tv[...,None])

Sharding: 8 cores = (batch b, L-half h).  Each core computes the full-L
scores for its batch (duplicated across the 2 cores of a batch, avoiding
any cross-core communication for the softmax) and the outputs for its own
L-half.  Inputs are rotated per-core so the own half is always tiles 0-3.

Layout: L on partitions (8 l-tiles of 128), D on the free dim.  The
Linear(32->768) weights ride as rhs of K=33 matmuls (bias folded in via a
ones-row in the lhsT).  A-side operands live on partitions 0-32 and V-side
on 64-96, so A/V matmul pairs land in disjoint PE row-groups and execute
concurrently.  TAQ/TVQ are a single ACT op each (per-partition scale=q,
bias=b).  Score reductions are fused mul+reduce (STT accum).  Softmax
skips max-subtraction (|scores| < 40 << 88).  tan = sin/cos with sin via
add_range_wrap into [-pi,pi] and a Cody-Waite cos exact near the poles.
"""

import numpy as np

B, L, D, F = 4, 1024, 768, 32
NCORES = 8
LT = 128          # l-tile size (partition dim)
NT = L // LT      # 8 l-tiles per batch
NT_HALF = NT // 2 # 4 own tiles
K1 = F + 1        # contraction with bias row
VOFF = 64         # partition offset of the V-side operands

PI = float(np.pi)
PIO2_HI = float(np.float32(np.pi / 2))
PIO2_LO = float(np.float64(np.pi / 2) - np.float64(np.float32(np.pi / 2)))

_CACHE = {}


def _build():
    if "nc" in _CACHE:
        return _CACHE["nc"]

    import concourse.bacc as bacc
    from concourse import bass_isa
    import concourse.tile as tile
    import concourse.mybir as mybir

    F32 = mybir.dt.float32
    AF = mybir.ActivationFunctionType
    ALU = mybir.AluOpType

    nc = bacc.Bacc()

    # ---- DRAM I/O (per-core shapes) ----
    d_t = nc.dram_tensor("t_rot", [L, D], F32, kind="ExternalInput")
    # av_pack rows: 0:33 = [A.T ; ones], 64:97 = [V.T ; ones]
    d_av = nc.dram_tensor("av_pack", [VOFF + K1, L], F32, kind="ExternalInput")
    # rhs_pack rows 0:33 = [Wa1.T|ba1 , Wa2.T|ba2], rows 64:97 = [Wv1.T|bv1 , Wv2.T|bv2]
    d_rhs = nc.dram_tensor("rhs_pack", [VOFF + K1, 2 * D], F32, kind="ExternalInput")
    d_wq = nc.dram_tensor("w_q", [VOFF + F, 1], F32, kind="ExternalInput")
    d_b = nc.dram_tensor("b_ab", [LT, 2], F32, kind="ExternalInput")
    d_oa = nc.dram_tensor("out_a", [L // 2, D], F32, kind="ExternalOutput")
    d_ov = nc.dram_tensor("out_v", [L // 2, D], F32, kind="ExternalOutput")

    t_view = d_t.rearrange("(n p) d -> p n d", p=LT)     # [128, 8, 768]
    oa_view = d_oa.rearrange("(n p) d -> p n d", p=LT)   # [128, 4, 768]
    ov_view = d_ov.rearrange("(n p) d -> p n d", p=LT)

    with tile.TileContext(nc) as tc:
        with (
            tc.tile_pool(name="consts", bufs=1) as consts,
            tc.tile_pool(name="keys", bufs=2) as keys,
            tc.tile_pool(name="vals", bufs=1) as vals,
            tc.tile_pool(name="vwork", bufs=3) as vwork,
            tc.tile_pool(name="ps", bufs=1, space="PSUM") as ps,
        ):
            # ---- inputs into SBUF ----
            # startup-critical DMAs first: tile-0's A-side matmul needs only
            # av rows 0:33 and rhs cols 0:512
            sb_av = consts.tile([VOFF + K1, L], F32, tag="sb_av")
            nc.sync.dma_start(out=sb_av[0:K1, :], in_=d_av[0:K1, :])
            sb_rhs = consts.tile([VOFF + K1, 2 * D], F32, tag="sb_rhs")
            nc.sync.dma_start(out=sb_rhs[0:K1, 0:512], in_=d_rhs[0:K1, 0:512])
            nc.sync.dma_start(out=sb_av[VOFF : VOFF + K1, :], in_=d_av[VOFF : VOFF + K1, :])
            nc.sync.dma_start(out=sb_rhs[VOFF : VOFF + K1, 0:512], in_=d_rhs[VOFF : VOFF + K1, 0:512])
            sb_wq = consts.tile([VOFF + F, 1], F32, tag="sb_wq")
            nc.sync.dma_start(out=sb_wq[:], in_=d_wq[:])
            nc.sync.dma_start(out=sb_rhs[0:K1, 512:D], in_=d_rhs[0:K1, 512:D])
            nc.sync.dma_start(out=sb_rhs[VOFF : VOFF + K1, 512:D], in_=d_rhs[VOFF : VOFF + K1, 512:D])
            sb_b = consts.tile([LT, 2], F32, tag="sb_b")
            nc.sync.dma_start(out=sb_b[:], in_=d_b[:])
            nc.sync.dma_start(out=sb_rhs[:, D : 2 * D], in_=d_rhs[:, D : 2 * D])
            t_all = consts.tile([LT, NT, D], F32, tag="t_all")
            nc.sync.dma_start(out=t_all[:, 0:1, :], in_=t_view[:, 0:1, :])
            nc.sync.dma_start(out=t_all[:, 1:4, :], in_=t_view[:, 1:4, :])
            nc.sync.dma_start(out=t_all[:, 4:NT, :], in_=t_view[:, 4:NT, :])

            sb_lo = consts.tile([LT, 1], F32, tag="sb_lo")
            nc.gpsimd.memset(sb_lo[:], PIO2_LO)
            # dummy first ACT op: pulls the tanh/exp table load to t=0 instead
            # of serializing it behind the first data-dependent activation
            warm = consts.tile([LT, 2], F32, tag="warm")
            nc.gpsimd.memset(warm[:], 0.0)
            nc.scalar.activation(out=warm[:, 1:2], in_=warm[:, 0:1], func=AF.Tanh)

            A0, A1 = 0, K1                  # a-side lhsT rows
            V0, V1 = VOFF, VOFF + K1        # v-side lhsT rows

            # ---- PE warmup: dependency-free dummy matmuls keep PE busy from
            # t=0 so the first real matmuls run at the warm rate (and warm the
            # HAM clock gate on real hardware) ----
            dmy = consts.tile([F, 64], F32, tag="dmy")
            nc.gpsimd.memset(dmy[:], 0.0)
            ps_d = ps.tile([64, 64], F32, tag="val", name="ps_d")
            for _k in range(20):
                nc.tensor.matmul(ps_d[:], dmy[:, 0:64], dmy[:, 0:64], start=True, stop=True)

            # ---- tile-0 key matmuls first: PE starts on the critical path
            # (taq needs sb_q only later, at its ACT op) ----
            ps_ak0 = ps.tile([LT, D], F32, tag="ak", name="ps_ak0")
            nc.tensor.matmul(ps_ak0[:, 0:512], sb_av[0:K1, 0:LT], sb_rhs[0:K1, 0:512], start=True, stop=True)
            nc.tensor.matmul(ps_ak0[:, 512:D], sb_av[0:K1, 0:LT], sb_rhs[0:K1, 512:D], start=True, stop=True)
            ps_vk0 = ps.tile([LT, D], F32, tag="vk", name="ps_vk0")
            nc.tensor.matmul(ps_vk0[:, 0:512], sb_av[VOFF:VOFF + K1, 0:LT], sb_rhs[VOFF:VOFF + K1, 0:512], start=True, stop=True)
            nc.tensor.matmul(ps_vk0[:, 512:D], sb_av[VOFF:VOFF + K1, 0:LT], sb_rhs[VOFF:VOFF + K1, 512:D], start=True, stop=True)

            # ---- qa/qv: per-l scalars via tiny (packed) matmuls ----
            ps_q = ps.tile([LT, 2 * NT], F32, tag="val")
            for i in range(NT):
                nc.tensor.matmul(
                    ps_q[:, 2 * i : 2 * i + 1],
                    sb_av[0:F, i * LT : (i + 1) * LT], sb_wq[0:F, :],
                    start=True, stop=True,
                )
                nc.tensor.matmul(
                    ps_q[:, 2 * i + 1 : 2 * i + 2],
                    sb_av[VOFF : VOFF + F, i * LT : (i + 1) * LT], sb_wq[VOFF : VOFF + F, :],
                    start=True, stop=True,
                )
            sb_q = consts.tile([LT, 2 * NT], F32, tag="sb_q")
            nc.vector.tensor_copy(out=sb_q[:], in_=ps_q[:])

            s_ta = consts.tile([LT, NT], F32, tag="s_ta")
            s_tv = consts.tile([LT, NT], F32, tag="s_tv")
            out_v_sb = consts.tile([LT, NT_HALF, D], F32, tag="out_v_sb")
            out_a_sb = consts.tile([LT, NT_HALF, D], F32, tag="out_a_sb")
            vvals, rss, rrs = [], [], []

            def emit_xa(j):
                """value-phase A-side: x_a2 matmuls + sin/cos argument prep
                (DVE frees the psum slot quickly)."""
                lsl = slice(j * LT, (j + 1) * LT)
                ps_xa = ps.tile([LT, D], F32, tag="val", name=f"ps_xa{j}")
                nc.tensor.matmul(ps_xa[:, 0:512], sb_av[A0:A1, lsl],
                                 sb_rhs[A0:A1, D : D + 512], start=True, stop=True)
                nc.tensor.matmul(ps_xa[:, 512:D], sb_av[A0:A1, lsl],
                                 sb_rhs[A0:A1, D + 512 : 2 * D], start=True, stop=True)
                rs = vals.tile([LT, D], F32, tag=f"rs{j}", name=f"rs{j}")
                nc.vector.add_range_wrap(out=rs[:], in_=ps_xa[:], shift=0.0, bound=PI, period=2 * PI)
                nax = vwork.tile([LT, D], F32, tag="nax", name=f"nax{j}")
                nc.vector.scalar_tensor_tensor(
                    out=nax[:], in0=rs[:], scalar=-1.0, in1=rs[:],
                    op0=ALU.mult, op1=ALU.min,
                )
                rr = vals.tile([LT, D], F32, tag=f"rr{j}", name=f"rr{j}")
                nc.vector.tensor_scalar(out=rr[:], in0=nax[:], scalar1=PIO2_HI,
                                        scalar2=None, op0=ALU.add)
                rss.append(rs)
                rrs.append(rr)

            def emit_xv(j):
                """value-phase V-side: x_v2 matmuls + vval tanh (ACT op is
                interleaved into the score-phase ACT stream; same table set)."""
                lsl = slice(j * LT, (j + 1) * LT)
                ps_xv = ps.tile([LT, D], F32, tag="val", name=f"ps_xv{j}")
                nc.tensor.matmul(ps_xv[:, 0:512], sb_av[V0:V1, lsl],
                                 sb_rhs[V0:V1, D : D + 512], start=True, stop=True)
                nc.tensor.matmul(ps_xv[:, 512:D], sb_av[V0:V1, lsl],
                                 sb_rhs[V0:V1, D + 512 : 2 * D], start=True, stop=True)
                vval = vals.tile([LT, D], F32, tag=f"vval{j}", name=f"vval{j}")
                nc.scalar.activation(out=vval[:], in_=ps_xv[:], func=AF.Tanh)
                vvals.append(vval)

            # ---- score phase over full L, with value work woven in ----
            for i in range(NT):
                lsl = slice(i * LT, (i + 1) * LT)
                # keypair psum [AKey | VKey]: bank0=a(512), bank1=a(256)+v(256),
                # bank2=v(512); issue order a1,v1,v2,a2 so the shared bank is
                # never written concurrently and A/V row-groups overlap.
                # split ak/vk psum tiles: AKey's tanh overlaps VKey's matmuls
                if i == 0:
                    ps_ak, ps_vk = ps_ak0, ps_vk0
                    akey = keys.tile([LT, D], F32, tag="akey", name="akey0")
                    nc.scalar.activation(out=akey[:, 0:512], in_=ps_ak[:, 0:512], func=AF.Tanh)
                    nc.scalar.activation(out=akey[:, 512:D], in_=ps_ak[:, 512:D], func=AF.Tanh)
                    split0 = True
                else:
                    ps_ak = ps.tile([LT, D], F32, tag="ak", name=f"ps_ak{i}")
                    nc.tensor.matmul(ps_ak[:, 0:512], sb_av[A0:A1, lsl], sb_rhs[A0:A1, 0:512], start=True, stop=True)
                    nc.tensor.matmul(ps_ak[:, 512:D], sb_av[A0:A1, lsl], sb_rhs[A0:A1, 512:D], start=True, stop=True)
                    ps_vk = ps.tile([LT, D], F32, tag="vk", name=f"ps_vk{i}")
                    nc.tensor.matmul(ps_vk[:, 0:512], sb_av[V0:V1, lsl], sb_rhs[V0:V1, 0:512], start=True, stop=True)
                    nc.tensor.matmul(ps_vk[:, 512:D], sb_av[V0:V1, lsl], sb_rhs[V0:V1, 512:D], start=True, stop=True)
                if i > 0:
                    akey = keys.tile([LT, D], F32, tag="akey")
                    nc.scalar.activation(out=akey[:], in_=ps_ak[:], func=AF.Tanh)
                taq = keys.tile([LT, D], F32, tag="taq")
                nc.scalar.activation(out=taq[:], in_=t_all[:, i, :], func=AF.Tanh,
                                     bias=sb_b[:, 0:1], scale=sb_q[:, 2 * i : 2 * i + 1])
                vkey = keys.tile([LT, D], F32, tag="vkey")
                nc.scalar.activation(out=vkey[:], in_=ps_vk[:], func=AF.Tanh)
                tvq = keys.tile([LT, D], F32, tag="tvq")
                nc.scalar.activation(out=tvq[:], in_=t_all[:, i, :], func=AF.Tanh,
                                     bias=sb_b[:, 1:2], scale=sb_q[:, 2 * i + 1 : 2 * i + 2])

                scr = keys.tile([LT, D], F32, tag="scr")
                nc.vector.scalar_tensor_tensor(
                    out=scr[:], in0=taq[:], scalar=1.0, in1=vkey[:],
                    op0=ALU.mult, op1=ALU.mult, accum_out=s_ta[:, i : i + 1],
                )
                scr2 = keys.tile([LT, D], F32, tag="scr2")
                nc.vector.scalar_tensor_tensor(
                    out=scr2[:], in0=tvq[:], scalar=1.0, in1=akey[:],
                    op0=ALU.mult, op1=ALU.mult, accum_out=s_tv[:, i : i + 1],
                )

                # weave value-phase work into the score stream
                if i % 2 == 1:
                    emit_xa(i // 2)
                elif i >= 2:
                    emit_xv(i // 2 - 1)
            emit_xv(3)

            # ---- softmax over all 1024 l's (no max subtraction; |s| < 40) ----
            e_ta = consts.tile([LT, NT], F32, tag="e_ta")
            e_tv = consts.tile([LT, NT], F32, tag="e_tv")
            rsum = consts.tile([LT, 2], F32, tag="rsum")
            nc.scalar.activation(out=e_ta[:], in_=s_ta[:], func=AF.Exp, accum_out=rsum[:, 0:1])
            exp_inst = nc.scalar.activation(out=e_tv[:], in_=s_tv[:], func=AF.Exp, accum_out=rsum[:, 1:2])
            zsum = consts.tile([LT, 2], F32, tag="zsum")
            nc.gpsimd.partition_all_reduce(zsum[:], rsum[:], channels=LT,
                                           reduce_op=bass_isa.ReduceOp.add)
            invzb = consts.tile([LT, 2], F32, tag="invzb")
            nc.vector.reciprocal(out=invzb[:], in_=zsum[:])
            ta_n = consts.tile([LT, NT_HALF], F32, tag="ta_n")
            nc.vector.tensor_scalar(out=ta_n[:], in0=e_ta[:, 0:NT_HALF],
                                    scalar1=invzb[:, 0:1], scalar2=None, op0=ALU.mult)
            tv_n = consts.tile([LT, NT_HALF], F32, tag="tv_n")
            nc.vector.tensor_scalar(out=tv_n[:], in0=e_tv[:, 0:NT_HALF],
                                    scalar1=invzb[:, 1:2], scalar2=None, op0=ALU.mult)

            # ---- value phase tail: scale vval, sin/cos, reciprocal, outputs ----
            for j in range(NT_HALF):
                nc.vector.tensor_scalar(out=out_v_sb[:, j, :], in0=vvals[j][:],
                                        scalar1=tv_n[:, j : j + 1], scalar2=None, op0=ALU.mult)
                if j == 1:
                    nc.sync.dma_start(out=ov_view[:, 0:2, :], in_=out_v_sb[:, 0:2, :])
            nc.sync.dma_start(out=ov_view[:, 2:4, :], in_=out_v_sb[:, 2:4, :])

            from concourse.tile import add_dep_helper
            for j in range(NT_HALF):
                sn = vwork.tile([LT, D], F32, tag="sn", bufs=4)
                i1 = nc.scalar.activation(out=sn[:], in_=rss[j][:], func=AF.Sin)
                cs = vwork.tile([LT, D], F32, tag="cs", bufs=4)
                i2 = nc.scalar.activation(out=cs[:], in_=rrs[j][:], func=AF.Sin, bias=sb_lo[:])
                # keep all Sin ops after the tanh/exp phase: one table switch
                add_dep_helper(i1.ins, exp_inst.ins, sync=False, reason="sin after exp (ACT table set)")
                add_dep_helper(i2.ins, exp_inst.ins, sync=False, reason="sin after exp (ACT table set)")
                rc = vwork.tile([LT, D], F32, tag="rc")
                nc.vector.reciprocal_approx_fast(out=rc[:], in_=cs[:])
                nc.vector.scalar_tensor_tensor(
                    out=out_a_sb[:, j, :], in0=sn[:], scalar=ta_n[:, j : j + 1], in1=rc[:],
                    op0=ALU.mult, op1=ALU.mult,
                )
                nc.sync.dma_start(out=oa_view[:, j : j + 1, :], in_=out_a_sb[:, j : j + 1, :])

    nc.finalize()
    _CACHE["nc"] = nc
    return nc


def _prep_in_maps(T, A, V, w_a, b_a, w_v, b_v,
                  W_aup1, b_aup1, W_aup2, b_aup2,
                  W_vup1, b_vup1, W_vup2, b_vup2):
    f32 = np.float32
    T = np.ascontiguousarray(np.asarray(T, f32))
    A = np.asarray(A, f32)
    V = np.asarray(V, f32)

    def aug_w(W, b):
        return np.concatenate([np.asarray(W, f32).T, np.asarray(b, f32)[None, :]], axis=0)

    rhs_pack = np.zeros((VOFF + K1, 2 * D), f32)
    rhs_pack[0:K1, 0:D] = aug_w(W_aup1, b_aup1)
    rhs_pack[0:K1, D : 2 * D] = aug_w(W_aup2, b_aup2)
    rhs_pack[VOFF : VOFF + K1, 0:D] = aug_w(W_vup1, b_vup1)
    rhs_pack[VOFF : VOFF + K1, D : 2 * D] = aug_w(W_vup2, b_vup2)

    w_q = np.zeros((VOFF + F, 1), f32)
    w_q[0:F, 0] = np.asarray(w_a, f32).reshape(F)
    w_q[VOFF : VOFF + F, 0] = np.asarray(w_v, f32).reshape(F)

    b_ab = np.empty((LT, 2), f32)
    b_ab[:, 0] = np.asarray(b_a, f32).reshape(())
    b_ab[:, 1] = np.asarray(b_v, f32).reshape(())

    ones = np.ones((1, L), f32)
    in_maps = []
    for c in range(NCORES):
        b, h = divmod(c, 2)
        rot = np.r_[np.arange(512 * h, L), np.arange(0, 512 * h)]
        av_pack = np.zeros((VOFF + K1, L), f32)
        av_pack[0:F] = A[b].T[:, rot]
        av_pack[F] = 1.0
        av_pack[VOFF : VOFF + F] = V[b].T[:, rot]
        av_pack[VOFF + F] = 1.0
        in_maps.append({
            "t_rot": np.ascontiguousarray(T[b][rot]),
            "av_pack": av_pack,
            "rhs_pack": rhs_pack,
            "w_q": w_q,
            "b_ab": b_ab,
        })
    return in_maps


def kernel(**inputs):
    from concourse.bass_utils import run_bass_kernel_spmd

    nc = _build()
    in_maps = _prep_in_maps(**inputs)
    res = run_bass_kernel_spmd(nc, in_maps, core_ids=list(range(NCORES)))

    out_a = np.empty((B, L, D), np.float32)
    out_v = np.empty((B, L, D), np.float32)
    for c in range(NCORES):
        b, h = divmod(c, 2)
        out_a[b, 512 * h : 512 * (h + 1)] = res.results[c]["out_a"]
        out_v[b, 512 * h : 512 * (h + 1)] = res.results[c]["out_v"]
    return out_a, out_v
